# revision 25
# baseline (speedup 1.0000x reference)
"""Trainium2 Bass/Tile kernel for a chained position-attention module (PAM).

Computation (per batch b):
  q,k,v   = 1x1-conv projections of x[b]   (C=64 channels, N=4096 positions)
  qg,kg   = projections of g[b]            (CG=32 channels)
  A  = softmax_rows(q^T k)                 (N,N)
  AG = softmax_rows(qg^T kg)               (N,N)
  GA = softmax_rows(A @ AG)                (N,N)
  out = gamma * (v @ GA^T) + x

Sharding: 8 cores = 4 batches x 2 query-row halves (2048 rows each).
Each core computes the full guide attention (UG) for its batch, then its
half of the chained attention rows.

All softmaxes are kept unnormalized on-chip; the row-sum reciprocals are
folded into downstream per-partition scales:
  - rg (guide row sums) folds into the transposed-U tile copies,
  - ru (attention row sums) folds into the ACT exp() scale of GE,
  - rga (chained row sums) folds into the final output scale.
"""

import sys

sys.path.insert(0, "/opt/trn_rl_repo")

import numpy as np

import concourse.bass as bass
import concourse.tile as tile
from concourse import bacc, mybir
from concourse.bass_utils import run_bass_kernel_spmd
from concourse.masks import make_identity

F32 = mybir.dt.float32
BF16 = mybir.dt.bfloat16
AF = mybir.ActivationFunctionType
ALU = mybir.AluOpType

B, C, CG, H, W = 4, 64, 32, 64, 64
N = H * W                 # 4096 positions
NCORES = 8
RH = N // 2               # 2048 query rows per core
NT = RH // 128            # 16 row tiles per core
MT = N // 128             # 32 contraction tiles
GRP = 4                   # row tiles per group (phases 2-4 pipelined per group)
NGRP = NT // GRP
CH1 = 512                 # free-dim chunk for phase 1/2 (energy matmuls)
CH3 = 512                 # free-dim chunk for phase 3 (chained matmul)
NCH1 = N // CH1
NCH3 = N // CH3

_compiled = None


def _build():
    nc = bacc.Bacc("TRN2", target_bir_lowering=False, debug=False,
                   num_devices=NCORES)

    xb_d = nc.dram_tensor("xb", [C, N], F32, kind="ExternalInput")
    xq_d = nc.dram_tensor("xq", [C, RH], F32, kind="ExternalInput")
    gb_d = nc.dram_tensor("gb", [CG, N], F32, kind="ExternalInput")
    wq_d = nc.dram_tensor("wq", [C, C], F32, kind="ExternalInput")
    wk_d = nc.dram_tensor("wk", [C, C], F32, kind="ExternalInput")
    wv_d = nc.dram_tensor("wv", [C, C], F32, kind="ExternalInput")
    wqg_d = nc.dram_tensor("wqg", [CG, CG], F32, kind="ExternalInput")
    wkg_d = nc.dram_tensor("wkg", [CG, CG], F32, kind="ExternalInput")
    bq_d = nc.dram_tensor("bq", [C, 1], F32, kind="ExternalInput")
    bk_d = nc.dram_tensor("bk", [C, 1], F32, kind="ExternalInput")
    bv_d = nc.dram_tensor("bv", [C, 1], F32, kind="ExternalInput")
    bqg_d = nc.dram_tensor("bqg", [CG, 1], F32, kind="ExternalInput")
    bkg_d = nc.dram_tensor("bkg", [CG, 1], F32, kind="ExternalInput")
    gam_d = nc.dram_tensor("gamma", [1, 1], F32, kind="ExternalInput")
    out_d = nc.dram_tensor("out", [C, RH], F32, kind="ExternalOutput")

    with tile.TileContext(nc) as tc:
        with (
            tc.tile_pool(name="dram", bufs=1, space="DRAM") as dramp,
            tc.tile_pool(name="const", bufs=1) as const,
            tc.tile_pool(name="small", bufs=4) as small,
            tc.tile_pool(name="psA", bufs=2, space="PSUM") as psA,
            tc.tile_pool(name="psB", bufs=3, space="PSUM") as psB,
            tc.tile_pool(name="psE", bufs=1, space="PSUM") as psE,
        ):
            ug_dram = dramp.tile([N, N], BF16)

            # ---- constants / identities ----
            idf = const.tile([128, 128], F32)
            make_identity(nc, idf)
            idb = const.tile([128, 128], BF16)
            make_identity(nc, idb)

            gam = const.tile([C, 1], F32)
            nc.sync.dma_start(out=gam, in_=gam_d[:, :].to_broadcast((C, 1)))

            biases = {}
            for name, dd, p in (("bq", bq_d, C), ("bk", bk_d, C),
                                ("bv", bv_d, C), ("bqg", bqg_d, CG),
                                ("bkg", bkg_d, CG)):
                t = const.tile([p, 1], F32, tag=name, name=name)
                nc.sync.dma_start(out=t, in_=dd[:, :])
                biases[name] = t

            # ---- load + transpose weights (lhsT = W^T, contraction on cin) --
            wT = {}
            for name, dd, p in (("wq", wq_d, C), ("wk", wk_d, C),
                                ("wv", wv_d, C), ("wqg", wqg_d, CG),
                                ("wkg", wkg_d, CG)):
                wnat = small.tile([p, p], F32, tag="wnat", name="wnat")
                nc.sync.dma_start(out=wnat, in_=dd[:, :])
                pt = psE.tile([128, 128], F32, tag="ef32", name="pt")
                nc.tensor.transpose(pt[:p, :p], wnat, idf[:p, :p])
                wt = const.tile([p, p], F32, tag=f"{name}T", name=f"{name}T")
                nc.vector.tensor_copy(out=wt, in_=pt[:p, :p])
                wT[name] = wt

            # persistent activations, row-group stacked for packed matmuls:
            # k2 = k duplicated on partitions 0-63 / 64-127;
            # q2 = even row-tiles on partitions 0-63, odd on 64-127.
            k2 = const.tile([2 * C, N], BF16)
            q2 = const.tile([2 * C, RH // 2], BF16)
            vT = const.tile([128, MT, C], BF16)

            rg_loc = const.tile([128, MT], F32)  # 1/rowsum of UG rows
            ru_all = const.tile([128, NT], F32)   # 1/rowsum of U (attention)
            rga_all = const.tile([128, NT], F32)  # 1/rowsum of GAu (chained)

            def project(dst, wt, src, bias_t, p, ncols):
                for ch in range(ncols // CH1):
                    sl = slice(ch * CH1, (ch + 1) * CH1)
                    ps = psA.tile([128, CH1], F32, name="ps")
                    nc.tensor.matmul(ps[:p, :], wt, src[:, sl])
                    nc.vector.tensor_scalar_add(
                        out=dst[:, sl], in0=ps[:p, :], scalar1=bias_t)

            # ---- phase 0 + 1 in scoped pools, freed before the main loop.
            # uT/u are opened FIRST so they occupy fresh SBUF: phase 2 can
            # then run concurrently with phase 1 (no released-zone reuse
            # deps); gau/gaT/ag open after phase 1 and recycle its space.
            _utp_cm = tc.tile_pool(name="uT", bufs=6)
            _up_cm = tc.tile_pool(name="u", bufs=3)
            utp = _utp_cm.__enter__()
            up = _up_cm.__enter__()
            with tc.tile_pool(name="ph1", bufs=1) as ph1p, \
                 tc.tile_pool(name="ug", bufs=3) as ugp:
                qg_sb = ph1p.tile([CG, N], BF16)
                kg_sb = ph1p.tile([CG, N], BF16)

                with tc.tile_pool(name="early", bufs=1) as early:
                    xb = early.tile([C, N], F32)
                    nc.sync.dma_start(out=xb, in_=xb_d[:, :])
                    xq = early.tile([C, RH], F32)
                    nc.sync.dma_start(out=xq, in_=xq_d[:, :])
                    gb = early.tile([CG, N], F32)
                    nc.sync.dma_start(out=gb, in_=gb_d[:, :])
                    v_bf = early.tile([C, N], BF16)

                    k_nat = early.tile([C, N], BF16)
                    q_nat = early.tile([C, RH], BF16)
                    project(k_nat, wT["wk"], xb, biases["bk"], C, N)
                    project(v_bf, wT["wv"], xb, biases["bv"], C, N)
                    project(q_nat, wT["wq"], xq, biases["bq"], C, RH)
                    project(qg_sb, wT["wqg"], gb, biases["bqg"], CG, N)
                    project(kg_sb, wT["wkg"], gb, biases["bkg"], CG, N)

                    # stack for row-group packing (SBUF->SBUF strided DMAs)
                    nc.sync.dma_start(out=k2[:C, :], in_=k_nat)
                    nc.sync.dma_start(out=k2[C:, :], in_=k_nat)
                    qv = q_nat.rearrange("c (p two f) -> c p two f",
                                         two=2, f=128)
                    q2v = q2.rearrange("c (p f) -> c p f", f=128)
                    nc.sync.dma_start(out=q2v[:C], in_=qv[:, :, 0, :])
                    nc.sync.dma_start(out=q2v[C:], in_=qv[:, :, 1, :])


                    # v^T tiles for the value aggregation
                    for mt in range(MT):
                        pt = psE.tile([128, 128], BF16, tag="eb16",
                                      bufs=2, name="pt")
                        nc.tensor.transpose(
                            pt[:, :C], v_bf[:, mt * 128:(mt + 1) * 128],
                            idb[:C, :C])
                        nc.vector.tensor_copy(out=vT[:, mt, :], in_=pt[:, :C])

                # ---- phase 1: guide attention AG -> DRAM (row-normalized) --
                for mt in range(MT):
                    ug_bf = ugp.tile([128, N], BF16, tag="ug", name="ug")
                    prt = small.tile([128, NCH1], F32, tag="prt1", name="prt1")
                    for ch in range(NCH1):
                        sl = slice(ch * CH1, (ch + 1) * CH1)
                        ps = psA.tile([128, CH1], F32, name="ps")
                        nc.tensor.matmul(
                            ps, qg_sb[:, mt * 128:(mt + 1) * 128], kg_sb[:, sl])
                        nc.scalar.activation(
                            out=ug_bf[:, sl], in_=ps, func=AF.Exp,
                            accum_out=prt[:, ch:ch + 1])
                    nc.vector.reduce_sum(
                        out=rg_loc[:, mt:mt + 1], in_=prt,
                        axis=mybir.AxisListType.X)
                    nc.vector.reciprocal(
                        out=rg_loc[:, mt:mt + 1], in_=rg_loc[:, mt:mt + 1])
                    nc.vector.tensor_scalar_mul(
                        out=ug_bf, in0=ug_bf, scalar1=rg_loc[:, mt:mt + 1])
                    nc.sync.dma_start(
                        out=ug_dram[mt * 128:(mt + 1) * 128, :], in_=ug_bf)

            # ---- phases 2-4 per group of GRP row tiles ----
            with (
                tc.tile_pool(name="gau", bufs=GRP) as gaup,
                tc.tile_pool(name="gaT", bufs=2) as gatp,
                tc.tile_pool(name="ag", bufs=2) as agp,
            ):
                for grp in range(NGRP):
                    nts = range(grp * GRP, (grp + 1) * GRP)

                    # phase 2: U rows (exp of energies) with nt pairs packed
                    # into PE row groups 0-1 / 2-3 (K=64 each), then
                    # PE-transpose into uT tiles.
                    uT_tiles = {}
                    u_tiles = {}
                    for p in (grp * GRP // 2, grp * GRP // 2 + 1):
                        psl = slice(p * 128, (p + 1) * 128)
                        u_a = up.tile([128, N], BF16, tag="u", name="u")
                        u_b = up.tile([128, N], BF16, tag="u", name="u")
                        prt_a = small.tile([128, NCH1], F32, tag="prt2",
                                           name="prt2")
                        prt_b = small.tile([128, NCH1], F32, tag="prt2b",
                                           name="prt2b")
                        for ch in range(NCH1):
                            sl = slice(ch * CH1, (ch + 1) * CH1)
                            ps_a = psA.tile([128, CH1], F32, name="ps")
                            ps_b = psA.tile([128, CH1], F32, name="ps")
                            nc.tensor.matmul(
                                ps_a, q2[:C, psl], k2[:C, sl],
                                tile_position=(0, 0))
                            nc.tensor.matmul(
                                ps_b, q2[C:, psl], k2[C:, sl],
                                tile_position=(64, 0))
                            nc.scalar.activation(
                                out=u_a[:, sl], in_=ps_a, func=AF.Exp,
                                accum_out=prt_a[:, ch:ch + 1])
                            nc.scalar.activation(
                                out=u_b[:, sl], in_=ps_b, func=AF.Exp,
                                accum_out=prt_b[:, ch:ch + 1])
                        for nt, u_bf, prt in ((2 * p, u_a, prt_a),
                                              (2 * p + 1, u_b, prt_b)):
                            nc.vector.reduce_sum(
                                out=ru_all[:, nt:nt + 1], in_=prt,
                                axis=mybir.AxisListType.X)
                            nc.vector.reciprocal(
                                out=ru_all[:, nt:nt + 1],
                                in_=ru_all[:, nt:nt + 1])
                            u_tiles[nt] = u_bf
                    for nt in nts:
                        uT = utp.tile([128, MT, 128], BF16, tag="uT",
                                      name="uT")
                        u_bf = u_tiles[nt]
                        for mt in range(MT):
                            pt = psE.tile([128, 128], BF16, tag="eb16",
                                          bufs=2, name="pt")
                            nc.tensor.transpose(
                                pt, u_bf[:, mt * 128:(mt + 1) * 128], idb)
                            nc.vector.tensor_copy(out=uT[:, mt, :], in_=pt)
                        uT_tiles[nt] = uT

                    # phase 3: GE' = U^T x UG (streamed), exp with ru folded
                    gau_tiles = {nt: gaup.tile([128, N], BF16, tag="gau",
                                               name="gau") for nt in nts}
                    gs_tiles = {nt: small.tile([128, NCH3], F32, tag="gsum",
                                               name="gsum") for nt in nts}
                    for ch2 in range(NCH3):
                        sl = slice(ch2 * CH3, (ch2 + 1) * CH3)
                        ag = agp.tile([128, MT, CH3], BF16, tag="ag", name="ag")
                        nc.sync.dma_start(
                            out=ag,
                            in_=ug_dram[:, sl].rearrange(
                                "(mt p) c -> p mt c", p=128))
                        for nt in nts:
                            gps = psB.tile([128, CH3], F32, name="gps")
                            for mt in range(MT):
                                nc.tensor.matmul(
                                    gps, uT_tiles[nt][:, mt, :], ag[:, mt, :],
                                    start=(mt == 0), stop=(mt == MT - 1))
                            nc.scalar.activation(
                                out=gau_tiles[nt][:, sl], in_=gps, func=AF.Exp,
                                scale=ru_all[:, nt:nt + 1],
                                accum_out=gs_tiles[nt][:, ch2:ch2 + 1])
                    for nt in nts:
                        nc.vector.reduce_sum(
                            out=rga_all[:, nt:nt + 1], in_=gs_tiles[nt],
                            axis=mybir.AxisListType.X)
                        nc.vector.reciprocal(
                            out=rga_all[:, nt:nt + 1],
                            in_=rga_all[:, nt:nt + 1])

                    # phase 4: transpose GA, aggregate values, residual
                    for nt in nts:
                        gaT = gatp.tile([128, MT, 128], BF16, tag="gaT",
                                        name="gaT")
                        for mt in range(MT):
                            pt = psE.tile([128, 128], BF16, tag="eb16",
                                          bufs=2, name="pt")
                            nc.tensor.transpose(
                                pt, gau_tiles[nt][:, mt * 128:(mt + 1) * 128],
                                idb)
                            nc.vector.tensor_copy(out=gaT[:, mt, :], in_=pt)
                        od = psE.tile([128, 128], F32, tag="ef32", name="od")[:, :C]
                        for mt in range(MT):
                            nc.tensor.matmul(
                                od, gaT[:, mt, :], vT[:, mt, :],
                                start=(mt == 0), stop=(mt == MT - 1))
                        ot = small.tile([128, C], F32, tag="ot", name="ot")
                        nc.vector.tensor_scalar_mul(
                            out=ot, in0=od, scalar1=rga_all[:, nt:nt + 1])
                        pt2 = psE.tile([128, 128], F32, tag="ef32", name="pt2")
                        nc.tensor.transpose(pt2[:C, :], ot, idf)
                        xr = small.tile([C, 128], F32, tag="xr", name="xr")
                        nc.sync.dma_start(
                            out=xr, in_=xq_d[:, nt * 128:(nt + 1) * 128])
                        res_t = small.tile([C, 128], F32, tag="res",
                                           name="res")
                        nc.vector.scalar_tensor_tensor(
                            out=res_t, in0=pt2[:C, :], scalar=gam, in1=xr,
                            op0=ALU.mult, op1=ALU.add)
                        nc.sync.dma_start(
                            out=out_d[:, nt * 128:(nt + 1) * 128], in_=res_t)

            _up_cm.__exit__(None, None, None)
            _utp_cm.__exit__(None, None, None)

    nc.compile()
    return nc


def _get_compiled():
    global _compiled
    if _compiled is None:
        _compiled = _build()
    return _compiled


def make_in_maps(x, g, Wq, bq, Wk, bk, Wv, bv, Wqg, bqg, Wkg, bkg, gamma):
    x = np.ascontiguousarray(x, dtype=np.float32)
    g = np.ascontiguousarray(g, dtype=np.float32)
    shared = {
        "wq": np.ascontiguousarray(Wq, np.float32),
        "wk": np.ascontiguousarray(Wk, np.float32),
        "wv": np.ascontiguousarray(Wv, np.float32),
        "wqg": np.ascontiguousarray(Wqg, np.float32),
        "wkg": np.ascontiguousarray(Wkg, np.float32),
        "bq": np.ascontiguousarray(bq, np.float32).reshape(C, 1),
        "bk": np.ascontiguousarray(bk, np.float32).reshape(C, 1),
        "bv": np.ascontiguousarray(bv, np.float32).reshape(C, 1),
        "bqg": np.ascontiguousarray(bqg, np.float32).reshape(CG, 1),
        "bkg": np.ascontiguousarray(bkg, np.float32).reshape(CG, 1),
        "gamma": np.ascontiguousarray(gamma, np.float32).reshape(1, 1),
    }
    in_maps = []
    for core in range(NCORES):
        b, half = core // 2, core % 2
        xb = x[b].reshape(C, N)
        m = dict(shared)
        m["xb"] = np.ascontiguousarray(xb)
        m["xq"] = np.ascontiguousarray(xb[:, half * RH:(half + 1) * RH])
        m["gb"] = np.ascontiguousarray(g[b].reshape(CG, N))
        in_maps.append(m)
    return in_maps


def kernel(x, g, Wq, bq, Wk, bk, Wv, bv, Wqg, bqg, Wkg, bkg, gamma):
    nc = _get_compiled()
    in_maps = make_in_maps(x, g, Wq, bq, Wk, bk, Wv, bv,
                           Wqg, bqg, Wkg, bkg, gamma)
    res = run_bass_kernel_spmd(nc, in_maps, list(range(NCORES)))
    out = np.empty((B, C, N), dtype=np.float32)
    for core in range(NCORES):
        b, half = core // 2, core % 2
        out[b][:, half * RH:(half + 1) * RH] = res.results[core]["out"]
    return out.reshape(B, C, H, W)
